# revision 49
# baseline (speedup 1.0000x reference)
"""Trainium2 Bass kernel for multi-head attention (B=2, S=2048, D=1024, H=16).

Sharding: 8 cores = 2 (batch, data-parallel) x 4 (head-groups, tensor-parallel).
Each core (b, g) handles batch b and heads [4g, 4g+4) (a 256-wide slice of the
model dim), computing a partial output contribution; the host sums the 4
head-group partials per batch and adds the output bias.

v2 layout/schedule changes over the first working version:
  - all DRAM tensors are host-prepacked partition-major so every DMA moves
    4-8KB contiguous per partition (the old rearrange APs made 512B packets
    and serialized ~10us of weight loads in front of the first matmul);
  - input DMAs ride the sync HWDGE ring only, weights + output ride the
    scalar ring only (no head-of-line blocking of activations behind
    output tiles);
  - the output projection and the qp projection for the next q-block are
    spread one matmul per k-tile through the attention steady state
    instead of 1.7us lumps that stalled the exp pipeline at q-block
    boundaries;
  - output is written bf16 (host accumulates the 4 partials in f32).
Steady state is ACT(exp)-bound at ~1.12us per k-tile; the first q-block is
PE-bound while the kp/vp/qp projections stream in under it.
"""

import os
import numpy as np
import ml_dtypes

import concourse.bass as bass
import concourse.bacc as bacc
import concourse.mybir as mybir
import concourse.tile as tile
from concourse.bass_utils import run_bass_kernel_spmd

F32 = mybir.dt.float32
BF16 = mybir.dt.bfloat16
AF = mybir.ActivationFunctionType

B, S, D = 2, 2048, 1024
H, DK = 16, 64
G = 4                  # head-groups (tensor parallel across cores)
DG = D // G            # 256 features per core
HPG = H // G           # 4 heads per core (2 row-packed pairs)
VEXT = HPG * (DK + 1)  # 260: per head [64 vp dims | 1 ones column]
P = 128
N_CORES = 8

_NC = None


def _build_program():
    nc = bacc.Bacc("TRN2", target_bir_lowering=False)
    # inputs, j/st-sliced and partition-major packed on the host:
    # qTj[j][p, t*512+s'] = q^T[t*128+p, j*512+s']  (t: contraction tile)
    qTj = [nc.dram_tensor(f"qTj{j}", [P, 8 * 512], BF16, kind="ExternalInput")
           for j in range(4)]
    kTj = [nc.dram_tensor(f"kTj{j}", [P, 8 * 512], BF16, kind="ExternalInput")
           for j in range(4)]
    # vTb[b][p, t*256+s'] = v^T[t*128+p, b*256+s']
    vTb = [nc.dram_tensor(f"vTb{b}", [P, 8 * 256], BF16, kind="ExternalInput")
           for b in range(8)]
    # weights, packed [p, t, m]
    wqT = nc.dram_tensor("wqT", [P, 8 * DG], BF16, kind="ExternalInput")
    wkT = nc.dram_tensor("wkT", [P, 8 * DG], BF16, kind="ExternalInput")
    wvm = nc.dram_tensor("wvm", [P, 8 * VEXT], BF16, kind="ExternalInput")
    wvb = nc.dram_tensor("wvb", [P, VEXT], BF16, kind="ExternalInput")
    woT = nc.dram_tensor("woT", [P, 2 * D], BF16, kind="ExternalInput")
    bqk = nc.dram_tensor("bqk", [P, 128], F32, kind="ExternalInput")
    out = nc.dram_tensor("out", [S, D], BF16, kind="ExternalOutput")

    with tile.TileContext(nc) as tc:
        _body(nc, tc, qTj, kTj, vTb, wqT, wkT, wvm, wvb, woT, bqk, out)
    nc.compile()
    return nc


def _body(nc, tc, qTj, kTj, vTb, wqT, wkT, wvm, wvb, woT, bqk, out):
    with (
        tc.tile_pool(name="consts", bufs=1) as consts,
        tc.tile_pool(name="persist", bufs=1) as persist,
        tc.tile_pool(name="stage", bufs=6) as stage,
        tc.tile_pool(name="etp", bufs=8) as etp,
        tc.tile_pool(name="small", bufs=3) as small,
        tc.tile_pool(name="outp", bufs=4) as outp,
        tc.tile_pool(name="psA", bufs=1, space="PSUM") as psA,
        tc.tile_pool(name="psQ", bufs=1, space="PSUM") as psQ,
        tc.tile_pool(name="psG", bufs=2, space="PSUM") as psG,
        tc.tile_pool(name="psC", bufs=1, space="PSUM") as psC,
    ):
        # --- ALL loads ride the sync HWDGE ring in strict deadline
        # order: a lone queue with big descriptors drains at ~430GB/s,
        # while two active queues round-robin per packet and the critical
        # first weights get starved behind the big activation stream ---
        wk_sb = consts.tile([P, 8, DG], BF16)
        nc.sync.dma_start(wk_sb[:], wkT[:].rearrange("p (t m) -> p t m", t=8))
        # bqk is padded to 128 f32/partition: a 16B/partition descriptor
        # stream clogs the DMA ring for microseconds
        bqk_sb = consts.tile([P, 128], F32)
        nc.sync.dma_start(bqk_sb[:], bqk[:])
        bq_sb = bqk_sb[:, 0:2]
        bk_sb = bqk_sb[:, 2:4]

        # warm the ACT exp table early so the ~2.7us load overlaps phase 1
        warm = consts.tile([1, 8], F32)
        nc.vector.memset(warm[:], 0.0)
        nc.scalar.activation(warm[:], warm[:], AF.Exp)

        # warm the PE HAM clock gate with throwaway matmuls so the
        # bootstrap projections run at 2.4GHz instead of the cold 1.2
        wdum = consts.tile([P, 512], BF16)
        nc.vector.memset(wdum[:], 0.25)

        # --- persistent activations ---
        qpT_sb = persist.tile([P, 2, S], BF16)   # [d%128, d-tile(=pair), s]
        kpT_sb = persist.tile([P, 2, S], BF16)
        vp_sb = persist.tile([P, 16, VEXT], BF16)  # [s%128, s-tile, 4*(64+1)]
        an_sb = persist.tile([P, 2, S], BF16)   # normalized attn output^T

        GRP = 2  # PSUM banks per exp group (one kt, both heads)

        def ps_alloc(n, i=[0]):
            i[0] += 1
            if i[0] % 2:
                return psA.tile([P, 512], F32, tag="a", name="ps_mm")[:, :n]
            return psG.tile([P, GRP * 512], F32, tag="g", name="gps")[:, :n]

        proj_xb = {}

        def proj_dma(which, j):
            src = qTj[j] if which == "q" else kTj[j]
            xb = stage.tile([P, 8, 512], BF16, tag="xb", name="xb")
            nc.sync.dma_start(xb[:], src[:].rearrange("p (t s) -> p t s", t=8))
            proj_xb[(which, j)] = xb

        def proj_half(which, w_sb, b_sb, dst, j, dt):
            # full 8-matmul projection of one (s-block j, feature-half dt)
            if (which, j) not in proj_xb:
                proj_dma(which, j)
            xb = proj_xb[(which, j)]
            ps = ps_alloc(512)
            for kt in range(8):
                nc.tensor.matmul(
                    ps[:],
                    lhsT=w_sb[:, kt, dt * P : (dt + 1) * P],
                    rhs=xb[:, kt, :],
                    start=(kt == 0),
                    stop=(kt == 7),
                )
            nc.vector.tensor_scalar_add(
                dst[:, dt, j * 512 : (j + 1) * 512], ps[:], b_sb[:, dt : dt + 1]
            )

        # spread variant: one contraction-step matmul at a time
        qp_spread = {}

        def qp_step(j, dt, kt):
            if (("q", j)) not in proj_xb:
                proj_dma("q", j)
            xb = proj_xb[("q", j)]
            key = (j, dt)
            if key not in qp_spread:
                qp_spread[key] = psQ.tile([P, 512], F32, tag="q", name="qp_ps")
            ps = qp_spread[key]
            nc.tensor.matmul(
                ps[:],
                lhsT=wq_sb[:, kt, dt * P : (dt + 1) * P],
                rhs=xb[:, kt, :],
                start=(kt == 0),
                stop=(kt == 7),
            )
            if kt == 7:
                nc.vector.tensor_scalar_add(
                    qpT_sb[:, dt, j * 512 : (j + 1) * 512],
                    ps[:],
                    bq_sb[:, dt : dt + 1],
                )
                del qp_spread[key]

        vtb_cache = {}

        def vp_block(st):
            # two s-tiles per DMA: the packed vTb block holds both
            st0 = st - st % 2
            if st0 not in vtb_cache:
                vtb2 = stage.tile([P, 8, 2 * P], BF16, tag="vtb", name="vtb")
                nc.sync.dma_start(
                    vtb2[:], vTb[st0 // 2][:].rearrange("p (t s) -> p t s", t=8)
                )
                vtb_cache[st0] = vtb2
            vtb = vtb_cache[st0]
            off = (st - st0) * P
            psv = ps_alloc(VEXT)
            for kt in range(8):
                nc.tensor.matmul(
                    psv[:],
                    lhsT=vtb[:, kt, off : off + P],
                    rhs=wv_sb[:, kt, :],
                    start=(kt == 0),
                    stop=(kt == 7),
                )
            nc.vector.tensor_tensor(
                vp_sb[:, st, :], psv[:], wvb_bc[:], mybir.AluOpType.add
            )

        # bootstrap, with loads emitted on the sync ring in deadline order
        # and throwaway matmuls warming the PE HAM clock gate (the engine
        # executes its stream in order, so the dummies run while the
        # first weight/input DMAs land and the projections then go at
        # the warm 2.4GHz)
        wps = psQ.tile([P, 512], F32, tag="q", name="wps")
        for _ in range(23):
            nc.tensor.matmul(wps[:], lhsT=wdum[:, :P], rhs=wdum[:],
                             start=True, stop=True)
        proj_dma("k", 0)
        wq_sb = consts.tile([P, 8, DG], BF16)
        nc.sync.dma_start(wq_sb[:], wqT[:].rearrange("p (t m) -> p t m", t=8))
        proj_dma("q", 0)
        # wv in two halves: the full 4160B/partition row splits into
        # pathological 416B descriptors, two 2080B DMAs stream cleanly
        wv_sb = consts.tile([P, 8, VEXT], BF16)
        wv_ap = wv_sb[:].rearrange("p t m -> p (t m)")
        nc.sync.dma_start(wv_ap[:, : 4 * VEXT], wvm[:, : 4 * VEXT])
        nc.sync.dma_start(wv_ap[:, 4 * VEXT :], wvm[:, 4 * VEXT :])
        wvb_bc = consts.tile([P, VEXT], BF16)
        nc.sync.dma_start(wvb_bc[:], wvb[:])
        proj_half("k", wk_sb, bk_sb, kpT_sb, 0, 0)
        # kp dt1 (pair1-only) hides under the wait for qTj0's arrival
        proj_half("k", wk_sb, bk_sb, kpT_sb, 0, 1)
        proj_half("q", wq_sb, bq_sb, qpT_sb, 0, 0)
        wo_sb = consts.tile([P, 2, D], BF16)
        nc.sync.dma_start(wo_sb[:], woT[:].rearrange("p (t o) -> p t o", t=2))

        # qb0/pair0 insertion schedule: vp blocks + kp halves (hard JIT
        # deadlines: kp j by kt 4j, vp st by kt st)
        fuse0 = {kt: [] for kt in range(16)}
        fuse0[0] = [("vp", 0), ("vp", 1)]
        for kt in range(14):
            fuse0[kt].append(("vp", kt + 2))
        for j in (1, 2, 3):
            fuse0[4 * j - 2].append(("kp", j, 0))
        for j in (1, 2, 3):
            fuse0[[6, 10, 13][j - 1]].append(("kp", j, 1))
        fuse0[12].append(("qp0", 0, 1))  # qp j0 dt1 for pair1 (full half)

        # steady-state insert slots: every insert is a SINGLE matmul so
        # it fits the ~200ns per-kt slack under the exp pipeline.  The
        # output-projection halves accumulate across two adjacent kts.
        # qp inserts have no cross-pair dependencies -> early slots;
        # d_mm reads an_sb rows the PREVIOUS pair's ~3.5us normalization
        # chain writes, so it gets the late half of the pass
        QP_SLOTS = (0, 1, 2, 3, 4, 5, 6, 7)          # 1 qp step per slot
        DU_SLOTS = {8: (0, 0), 9: (0, 1), 10: (1, 0), 11: (1, 1),
                    12: (2, 0), 13: (2, 1), 14: (3, 0), 15: (3, 1)}

        osb_cache = {}
        dps_live = {}

        def d_mm(qb, qt, o, p2, ring):
            # one matmul of one (q-tile, out-half) of the output
            # projection; p2==1 closes the accumulation, casts and (for
            # o==1) flushes the q-tile row to DRAM
            q0 = qb * 512 + qt * P
            if p2 == 0:
                dps_live[(qt, o)] = psA.tile([P, 512], F32, tag="a",
                                             name="dps")
            dps = dps_live[(qt, o)]
            nc.tensor.matmul(
                dps[:],
                lhsT=an_sb[:, p2, q0 : q0 + P],
                rhs=wo_sb[:, p2, o * 512 : (o + 1) * 512],
                start=(p2 == 0),
                stop=(p2 == 1),
            )
            if p2 == 0:
                return
            del dps_live[(qt, o)]
            if qt not in osb_cache:
                osb_cache[qt] = outp.tile([P, D], BF16, tag="o", name="osb")
            osb = osb_cache[qt]
            nc.vector.tensor_copy(osb[:, o * 512 : (o + 1) * 512], dps[:])
            if o == 1:
                ring.dma_start(out[q0 : q0 + P, :], osb[:])
                del osb_cache[qt]

        def attention_pass(qb, pair, inserts, prefetches):
            qs = slice(qb * 512, (qb + 1) * 512)
            cc = psC.tile([DK + 1, 1024], F32, tag="c", name="cc")
            c_ps = [cc[:, :512], cc[:, 512:]]
            last = qb == 3 and pair == 1
            et_last = None
            for kt in range(16):
                if kt in prefetches:
                    prefetches[kt]()
                gps = psG.tile([P, GRP * 512], F32, tag="g", name="gps")
                for hh in range(2):
                    hp = slice(hh * DK, (hh + 1) * DK)
                    nc.tensor.matmul(
                        gps[:, hh * 512 : (hh + 1) * 512],
                        lhsT=kpT_sb[hp, pair, kt * P : (kt + 1) * P],
                        rhs=qpT_sb[hp, pair, qs],
                        start=True,
                        stop=True,
                    )
                et = etp.tile([P, GRP * 512], BF16, tag="e", name="et")
                et_last = et
                nc.scalar.activation(
                    et[:], gps[:], AF.Exp, scale=1.0 / np.sqrt(DK)
                )
                for hh in range(2):
                    h = 2 * pair + hh
                    nc.tensor.matmul(
                        c_ps[hh],
                        lhsT=vp_sb[:, kt, h * (DK + 1) : (h + 1) * (DK + 1)],
                        rhs=et[:, hh * 512 : (hh + 1) * 512],
                        start=(kt == 0),
                        stop=(kt == 15),
                    )
                if kt in inserts:
                    inserts[kt]()
            # normalization: one fast DVE copy releases the accumulator;
            # the very last pair's norm is chunked into the tail loop
            if last:
                return cc, et_last
            csb = small.tile([DK + 1, 1024], F32, tag="csb")
            nc.vector.tensor_copy(csb[:], cc[:])
            csrc = csb
            for hh in range(2):
                cs = slice(hh * 512, (hh + 1) * 512)
                rsum = small.tile([1, 512], F32, tag="rsum")
                nc.vector.tensor_copy(rsum[:], csrc[DK : DK + 1, cs])
                rinv = small.tile([1, 512], F32, tag="rinv")
                nc.vector.reciprocal_approx_fast(rinv[:], rsum[:])
                rbc = small.tile([DK, 512], F32, tag="rbc")
                nc.gpsimd.partition_broadcast(rbc[:], rinv[:])
                nc.vector.tensor_tensor(
                    an_sb[hh * DK : (hh + 1) * DK, pair, qs],
                    csrc[:DK, cs],
                    rbc[:],
                    mybir.AluOpType.mult,
                )

        # --- qb0: projections stream in under the attention pipeline ---
        ins0 = {}
        for kt, items in fuse0.items():
            def run(items=items):
                for item in items:
                    if item[0] == "vp":
                        vp_block(item[1])
                    elif item[0] == "kp":
                        proj_half("k", wk_sb, bk_sb, kpT_sb, item[1], item[2])
                    else:
                        proj_half("q", wq_sb, bq_sb, qpT_sb, item[1], item[2])
            if items:
                ins0[kt] = run
        attention_pass(0, 0, ins0, {8: lambda: proj_dma("q", 1)})

        # qb0/pair1: qp j1, one contraction step per kt (dt0 then dt1)
        ins = {kt: (lambda kt=kt: qp_step(1, kt // 8, kt % 8))
               for kt in range(16)}
        attention_pass(0, 1, ins, {0: lambda: proj_dma("q", 2)})

        # --- steady q-blocks ---
        for qb in (1, 2, 3):
            for pair in (0, 1):
                inserts = {}
                if qb < 3:
                    j = qb + 1
                    for si, kt in enumerate(QP_SLOTS):
                        inserts[kt] = (lambda j=j, dt=pair, si=si:
                                       qp_step(j, dt, si))
                for kt, (half, p2) in DU_SLOTS.items():
                    qt = 2 * pair + half // 2
                    o = half % 2
                    def run(qt=qt, o=o, p2=p2, qb=qb):
                        d_mm(qb - 1, qt, o, p2, nc.sync)
                    inserts[kt] = run
                prefetches = {}
                if qb == 1 and pair == 1:
                    prefetches[0] = lambda: proj_dma("q", 3)
                ret = attention_pass(qb, pair, inserts, prefetches)

        # --- tail for qb3/pair1: chunked norm + output projection ---
        cc3, et3 = ret
        # throwaway matmuls anchored on the last exp tile: they become
        # runnable exactly when the tail starts, keeping the PE HAM
        # clock warm through the first normalization chunk
        wps3 = psQ.tile([P, 512], F32, tag="q", name="wps3")
        for _ in range(12):
            nc.tensor.matmul(wps3[:], lhsT=et3[:, :P], rhs=et3[:, :512],
                             start=True, stop=True)
        # per 128-column chunk: rowsum copy / reciprocal / broadcast /
        # two head multiplies, then immediately that q-tile's output
        # projection (psG pool is free after the last exp); casts ride
        # the now-idle ACT engine, DMAs alternate rings
        q0b = 3 * 512
        rinvs, rbcs = [], []
        for qt in range(4):
            cs = slice(qt * P, (qt + 1) * P)
            rsum = small.tile([1, 2 * P], F32, tag="rsum")
            nc.vector.tensor_copy(
                rsum[:].rearrange("r (h q) -> r h q", h=2),
                cc3[DK : DK + 1, :].rearrange("r (h q) -> r h q", h=2)[:, :, cs],
            )
            rinv = small.tile([1, 2 * P], F32, tag="rinv")
            nc.vector.reciprocal_approx_fast(rinv[:], rsum[:])
            rinvs.append(rinv)
        for qt in range(4):
            rbc = small.tile([DK, 2 * P], F32, tag="rbc")
            nc.gpsimd.partition_broadcast(rbc[:], rinvs[qt][:])
            rbcs.append(rbc)
        for qt in range(4):
            rbc = rbcs[qt]
            q0 = q0b + qt * P
            for hh in range(2):
                nc.vector.tensor_tensor(
                    an_sb[hh * DK : (hh + 1) * DK, 1, q0 : q0 + P],
                    cc3[:DK, hh * 512 + qt * P : hh * 512 + (qt + 1) * P],
                    rbc[:, hh * P : (hh + 1) * P],
                    mybir.AluOpType.mult,
                )
            gps3 = psG.tile([P, GRP * 512], F32, tag="g", name="gps")
            for o in range(2):
                for p2 in range(2):
                    nc.tensor.matmul(
                        gps3[:, o * 512 : (o + 1) * 512],
                        lhsT=an_sb[:, p2, q0 : q0 + P],
                        rhs=wo_sb[:, p2, o * 512 : (o + 1) * 512],
                        start=(p2 == 0),
                        stop=(p2 == 1),
                    )
            osb3 = outp.tile([P, D], BF16, tag="o", name="osb3")
            nc.scalar.activation(osb3[:], gps3[:], AF.Copy)
            (nc.scalar if qt % 2 else nc.sync).dma_start(
                out[q0 : q0 + P, :], osb3[:])


def _get_program():
    global _NC
    if _NC is None:
        _NC = _build_program()
    return _NC


def _make_in_maps(v, k, q, Wv, bv, Wk, bk, Wq, bq, Wo, bo):
    f32 = np.float32
    bf16 = ml_dtypes.bfloat16

    def pack_in(x):
        # x: [S, D] activation -> xT [D, S] -> j-sliced [4][128, 8*512]
        xT = np.ascontiguousarray(x.T).astype(bf16)          # [1024, 2048]
        a = xT.reshape(8, P, 4, 512).transpose(2, 1, 0, 3)    # [j, p, t, s']
        return [np.ascontiguousarray(a[j].reshape(P, 8 * 512)) for j in range(4)]

    def pack_v(x):
        xT = np.ascontiguousarray(x.T).astype(bf16)
        a = xT.reshape(8, P, 8, 256).transpose(2, 1, 0, 3)    # [b, p, t, s']
        return [np.ascontiguousarray(a[b].reshape(P, 8 * 256)) for b in range(8)]

    def pack_w(w):
        # w: [1024, M] -> [128, 8*M] with [p, t, m]
        M = w.shape[1]
        a = w.reshape(8, P, M).transpose(1, 0, 2)
        return np.ascontiguousarray(a.reshape(P, 8 * M))

    qTs = [pack_in(q[b]) for b in range(B)]
    kTs = [pack_in(k[b]) for b in range(B)]
    vTs = [pack_v(v[b]) for b in range(B)]

    per_group = []
    for g in range(G):
        gs = slice(g * DG, (g + 1) * DG)
        wqT = pack_w(np.ascontiguousarray(Wq[gs, :].T).astype(bf16))
        wkT = pack_w(np.ascontiguousarray(Wk[gs, :].T).astype(bf16))
        wvm = np.zeros((D, VEXT), dtype=f32)
        wvb = np.zeros((1, VEXT), dtype=f32)
        for h in range(HPG):
            cs = slice(h * (DK + 1), h * (DK + 1) + DK)
            rows = slice(g * DG + h * DK, g * DG + (h + 1) * DK)
            wvm[:, cs] = Wv[rows, :].T
            wvb[0, cs] = bv[rows]
            wvb[0, h * (DK + 1) + DK] = 1.0
        wvm = pack_w(wvm.astype(bf16))
        wvb_bc = np.ascontiguousarray(
            np.broadcast_to(wvb.astype(bf16), (P, VEXT)))
        woT = np.ascontiguousarray(Wo[:, gs].T).astype(bf16)  # [256, 1024]
        woTp = np.ascontiguousarray(
            woT.reshape(2, P, D).transpose(1, 0, 2).reshape(P, 2 * D))
        bqk = np.zeros((P, 128), dtype=f32)
        bqk[:, 0] = bq[gs][:P]
        bqk[:, 1] = bq[gs][P:]
        bqk[:, 2] = bk[gs][:P]
        bqk[:, 3] = bk[gs][P:]
        per_group.append(dict(wqT=wqT, wkT=wkT, wvm=wvm, wvb=wvb_bc,
                              woT=woTp, bqk=bqk))

    in_maps = []
    for c in range(N_CORES):
        b, g = c // G, c % G
        m = dict(**per_group[g])
        for j in range(4):
            m[f"qTj{j}"] = qTs[b][j]
            m[f"kTj{j}"] = kTs[b][j]
        for sb in range(8):
            m[f"vTb{sb}"] = vTs[b][sb]
        in_maps.append(m)
    return in_maps


def _gather(results, bo):
    out = np.zeros((B, S, D), dtype=np.float32)
    for c in range(N_CORES):
        b = c // G
        out[b] += np.asarray(results[c]["out"], dtype=np.float32)
    out += bo.astype(np.float32)
    return out


def run(v, k, q, Wv, bv, Wk, bk, Wq, bq, Wo, bo, trace=False):
    nc = _get_program()
    in_maps = _make_in_maps(v, k, q, Wv, bv, Wk, bk, Wq, bq, Wo, bo)
    res = run_bass_kernel_spmd(
        nc, in_maps, core_ids=list(range(N_CORES)), trace=trace
    )
    return _gather(res.results, np.asarray(bo)), res


def kernel(v, k, q, Wv, bv, Wk, bk, Wq, bq, Wo, bo):
    args = [np.asarray(x, dtype=np.float32)
            for x in (v, k, q, Wv, bv, Wk, bk, Wq, bq, Wo, bo)]
    out, _ = run(*args, trace=bool(int(os.environ.get("MHA_TRACE", "0"))))
    return out
